# revision 31
# baseline (speedup 1.0000x reference)
"""DenseRadiusGraph (B=16 graphs x N=2048 nodes, D=3, cutoff=10, K=32)
on 8 Trainium2 NeuronCores.

Strategy
--------
Shard over the graph axis: each of the 8 cores handles 2 graphs; no
cross-core communication.

Per core (device), for each 128-row block of a graph's [N, N] distance
matrix:
  - A K=15 feature GEMM (hi/lo-split bf16: exact bf16 x bf16 products,
    fp32 PSUM accumulate) on the tensor engine computes
        C + z,   z = ZOFF - d2(i, j)/4,   C = 2^23
    directly into PSUM. z lies in [2^14, 2^15), so the fp32 PSUM store
    at 2^23 scale rounds z to the integer grid for free, leaving the
    low 9 mantissa bits clear.
  - One ACT op subtracts C; a gpsimd/DVE column-split add embeds the
    column index j%512 * 2^-9 into the low bits (15+9 = 24 mantissa
    bits: exact).
  - The DVE max8 op extracts, per 512-wide column chunk, the top-8
    values (= 8 smallest d2, index embedded). The radius-graph data
    (cutoff 10 in a 100-box) has ~9 in-cutoff neighbours per row, so a
    chunk's top-8 usually contains every in-cutoff neighbour of that
    chunk; saturated chunks (8th-best still inside the cutoff margin,
    ~830 rows here) are detected on the host and those rows re-solved
    exactly there.

Host post-pass: decode candidate columns, exact re-score with bit-exact
replication of the reference arithmetic (XLA CPU computes the einsum as
an fp32 FMA chain over d=0,1,2, emulated in float64), final top-K +
cutoff + edge emission, plus the exact row-level fallback for saturated
chunks. The O(N^2) distance work and the top-k scan run on device; the
host only re-ranks 64 candidates/row and formats edges.
"""

import numpy as np

import concourse.bass as bass
import concourse.tile as tile
from concourse import bacc, mybir
from concourse.bass_utils import run_bass_kernel_spmd

B = 16          # graphs
N = 2048        # nodes per graph
D = 3
K = 32          # max neighbours emitted
CUTOFF = np.float32(10.0)
NCORES = 8
GPC = B // NCORES            # graphs per core
RB = 128                     # rows per block
NRB = N // RB                # row blocks per graph
CHW = 512                    # selection chunk width
NCH = N // CHW               # chunks per row
CAND = NCH * 8               # candidates per row

GEMM_BF16 = True             # hi/lo-split bf16 GEMM (4x PE throughput)

# The GEMM computes z = ZOFF - d2/4 (features scaled by exact 0.25), so z
# sits in [2^14, 2^15): 15 integer bits + 9 index bits = 24 = fp32 mantissa.
ZOFF = np.float32(24576.0)   # bf16-exact binade offset
D2SCALE = np.float32(4.0)    # d2 = (ZOFF - z) * D2SCALE
CMAGIC = np.float32(8388608.0)    # 2^23: baked into the GEMM so PSUM fp32
                                  # storage rounds z to the integer grid
NFEAT = 15
JEPS = np.float32(1.0 / 512.0)   # index embedding step (2^-9)
SAT_MARGIN = np.float32(16.0)    # d2 slack for saturation/noise margins

DIAG_MASK = False
SPLIT = 1472                 # posc-add column split: [0,SPLIT) gpsimd, rest DVE
NACT = 1                     # ACT -C op split count

_cached = None
last_exec_ns = None


def _build():
    """Build + compile the per-core Bass program (same NEFF for all cores)."""
    nc = bacc.Bacc("TRN2", target_bir_lowering=False, debug=False)
    f32 = mybir.dt.float32
    fdt = mybir.dt.bfloat16

    U = nc.dram_tensor("U", [GPC, NFEAT, N], fdt, kind="ExternalInput")
    V = nc.dram_tensor("V", [GPC, NFEAT, N], fdt, kind="ExternalInput")
    POSC = nc.dram_tensor("POSC", [1, N], f32, kind="ExternalInput")
    vals_out = nc.dram_tensor(
        "vals_out", [GPC, NRB, RB, CAND], f32, kind="ExternalOutput"
    )

    with tile.TileContext(nc) as tc:
        with (
            tc.tile_pool(name="feat", bufs=2) as feat,
            tc.tile_pool(name="posp", bufs=1) as posp,
            tc.tile_pool(name="ps", bufs=2, space="PSUM") as ps,
            tc.tile_pool(name="sb", bufs=2) as sbp,
            tc.tile_pool(name="outp", bufs=3) as outp,
        ):
            posc = posp.tile([RB, N], f32)
            nc.sync.dma_start(posc[:], bass.AP(POSC, 0, [[0, RB], [1, N]]))
            nbias = posp.tile([RB, 1], f32)
            nc.vector.memset(nbias[:], float(-CMAGIC))
            for g in range(GPC):
                u_t = feat.tile([NFEAT, N], fdt, tag="u")
                v_t = feat.tile([NFEAT, N], fdt, tag="v")
                nc.sync.dma_start(u_t[:], U.ap()[g])
                nc.sync.dma_start(v_t[:], V.ap()[g])
                for rb in range(NRB):
                    psum = ps.tile([RB, N], f32)
                    for cb in range(N // 512):
                        nc.tensor.matmul(
                            psum[:, cb * 512:(cb + 1) * 512],
                            u_t[:, rb * RB:(rb + 1) * RB],
                            v_t[:, cb * 512:(cb + 1) * 512],
                        )
                    # PSUM holds C + z already quantized to the integer grid
                    # (fp32 storage at 2^23 scale); ACT subtracts C back.
                    zq = sbp.tile([RB, N], f32, tag="zq")
                    for ab in range(NACT):
                        aw = N // NACT
                        nc.scalar.activation(
                            zq[:, ab * aw:(ab + 1) * aw],
                            psum[:, ab * aw:(ab + 1) * aw],
                            mybir.ActivationFunctionType.Identity,
                            bias=nbias[:, 0:1], scale=1.0,
                        )
                    v_sel = sbp.tile([RB, N], f32, tag="vsel")
                    nc.gpsimd.tensor_tensor(
                        out=v_sel[:, 0:SPLIT], in0=zq[:, 0:SPLIT],
                        in1=posc[:, 0:SPLIT], op=mybir.AluOpType.add,
                    )
                    nc.vector.tensor_tensor(
                        out=v_sel[:, SPLIT:N], in0=zq[:, SPLIT:N],
                        in1=posc[:, SPLIT:N], op=mybir.AluOpType.add,
                    )
                    if DIAG_MASK:
                        nc.gpsimd.affine_select(
                            v_sel[:, rb * RB:(rb + 1) * RB],
                            v_sel[:, rb * RB:(rb + 1) * RB],
                            pattern=[[1, RB]],
                            compare_op=mybir.AluOpType.not_equal,
                            fill=0.0, base=0, channel_multiplier=-1,
                        )
                    vals_t = outp.tile([RB, CAND], f32, tag="vals")
                    for c in range(NCH):
                        nc.vector.max(
                            out=vals_t[:, c * 8:(c + 1) * 8],
                            in_=v_sel[:, c * CHW:(c + 1) * CHW],
                        )
                    nc.sync.dma_start(vals_out.ap()[g, rb], vals_t[:])
    nc.compile()
    return nc


def _get_nc():
    global _cached
    if _cached is None:
        _cached = _build()
    return _cached


def _sq_ref(x):
    """|x|^2 per row, bit-exact to jnp.sum(p*p, -1) on XLA CPU
    (rounded products, sequential adds)."""
    p0 = x[:, 0] * x[:, 0]
    p1 = x[:, 1] * x[:, 1]
    p2 = x[:, 2] * x[:, 2]
    return (p0 + p1) + p2


def _dot_fma(xr, xc):
    """fp32 FMA-chain dot over d=0,1,2, bit-exact to XLA CPU einsum.
    Emulated via float64 (exact product + add, one fp32 rounding)."""
    a = np.float32(xr[..., 0].astype(np.float64) * xc[..., 0].astype(np.float64))
    a = np.float32(
        xr[..., 1].astype(np.float64) * xc[..., 1].astype(np.float64)
        + a.astype(np.float64)
    )
    a = np.float32(
        xr[..., 2].astype(np.float64) * xc[..., 2].astype(np.float64)
        + a.astype(np.float64)
    )
    return a


def _ref_dist(xr, xc, sq_r, sq_c):
    """Reference-bit dist for row points xr [..., 3] vs cols xc [..., 3]."""
    dot = _dot_fma(xr, xc)
    d2 = (sq_r + sq_c) - np.float32(2.0) * dot
    return np.sqrt(np.maximum(d2, np.float32(0.0)))


def _make_in_maps(p, sq):
    import ml_dtypes

    bf16 = ml_dtypes.bfloat16
    posc = ((np.arange(N, dtype=np.float32) % CHW) * JEPS).astype(np.float32)
    in_maps = []
    for core in range(NCORES):
        Uc = np.empty((GPC, NFEAT, N), bf16)
        Vc = np.empty((GPC, NFEAT, N), bf16)
        for g in range(GPC):
            b = core * GPC + g
            x = p[b]                                   # [N, 3] f32
            xh = x.astype(bf16)
            xl = (x - xh.astype(np.float32)).astype(bf16)
            sqh = sq[b].astype(bf16)
            sql = (sq[b] - sqh.astype(np.float32)).astype(bf16)
            # U rows carry the exact pow2 scale 2/D2SCALE = 0.5 (and
            # 1/D2SCALE for the sq rows) so PSUM = C + ZOFF - d2/D2SCALE
            hsc = np.float32(2.0) / D2SCALE
            Uc[g, 0:3] = (hsc * xh.astype(np.float32)).astype(bf16).T
            Uc[g, 3:6] = Uc[g, 0:3]
            Uc[g, 6:9] = (hsc * xl.astype(np.float32)).astype(bf16).T
            Uc[g, 9] = (-sqh.astype(np.float32) / D2SCALE).astype(bf16)
            Uc[g, 10] = (-sql.astype(np.float32) / D2SCALE).astype(bf16)
            Uc[g, 11] = bf16(-1.0 / float(D2SCALE))
            Uc[g, 12] = bf16(-1.0 / float(D2SCALE))
            Uc[g, 13] = bf16(float(ZOFF))
            Uc[g, 14] = bf16(float(CMAGIC))
            Vc[g, 0:3] = xh.T
            Vc[g, 3:6] = xl.T
            Vc[g, 6:9] = xh.T
            Vc[g, 9] = bf16(1.0)
            Vc[g, 10] = bf16(1.0)
            Vc[g, 11] = sqh
            Vc[g, 12] = sql
            Vc[g, 13] = bf16(1.0)
            Vc[g, 14] = bf16(1.0)
        in_maps.append({"U": Uc, "V": Vc, "POSC": posc[None, :].copy()})
    return in_maps


def kernel(pos, batch):
    pos = np.ascontiguousarray(np.asarray(pos), dtype=np.float32)
    nc = _get_nc()

    p = pos.reshape(B, N, D)
    sq = np.stack([_sq_ref(p[b]) for b in range(B)])          # [B, N] f32

    res = run_bass_kernel_spmd(nc, _make_in_maps(p, sq), core_ids=list(range(NCORES)))
    global last_exec_ns
    last_exec_ns = res.exec_time_ns

    vals = np.empty((B, N, CAND), np.float32)
    for core in range(NCORES):
        r = res.results[core]["vals_out"]                     # [GPC, NRB, RB, CAND]
        for g in range(GPC):
            vals[core * GPC + g] = r[g].reshape(N, CAND)

    # decode candidate columns from the index-embedded values
    v64 = vals.astype(np.float64)
    j = np.rint((v64 % 1.0) * CHW).astype(np.int32)
    chunk_base = (np.arange(CAND, dtype=np.int32) // 8) * CHW
    cand = j + chunk_base                                     # [B, N, CAND]

    # saturated chunk detection: 8th-best of a chunk still within the
    # cutoff margin means the chunk may have had >8 in-cutoff columns
    zq = np.floor(v64)                                        # [B, N, CAND]
    d2_approx = (float(ZOFF) - zq) * float(D2SCALE)
    sat = (d2_approx <= float(CUTOFF) ** 2 + float(SAT_MARGIN))
    # slots 7, 15, ... are each chunk's 8th-best
    sat_rows = np.nonzero(sat[:, :, 7::8].any(-1))

    return _host_finish(p, sq, cand, sat_rows)


def _host_finish(p, sq, cand, sat_rows):
    """Exact re-score of candidates with reference-bit arithmetic, final
    top-K + cutoff + edge emission, exact fallback for saturated rows."""
    rows_l = np.arange(N, dtype=np.int32)[None, :, None]       # graph-local row

    bidx = np.arange(B)[:, None, None]
    xr = p[:, :, None, :]                                      # [B, N, 1, 3]
    xc = p[bidx, cand]                                         # [B, N, CAND, 3]
    dist = _ref_dist(xr, xc, sq[:, :, None], sq[bidx, cand])
    dist = np.where(cand == rows_l, np.float32(np.inf), dist)  # drop self-edge

    # sort each row's candidates by (dist, col) ascending — matches
    # jax.lax.top_k's stable lowest-index tie-break
    order = np.lexsort((cand, dist.view(np.uint32)), axis=-1)[:, :, :K]
    col_k = np.take_along_axis(cand, order, axis=-1)           # [B, N, K]
    dist_k = np.take_along_axis(dist, order, axis=-1)

    # exact fallback: saturated rows are re-solved against all N columns
    sb, sr = sat_rows
    if sb.size:
        frow = _ref_dist(p[sb, sr, None, :], p[sb], sq[sb, sr, None], sq[sb])
        frow[np.arange(sb.size), sr] = np.float32(np.inf)      # self
        forder = np.lexsort(
            (np.broadcast_to(np.arange(N, dtype=np.int32), frow.shape),
             frow.view(np.uint32)), axis=-1)[:, :K]
        col_k[sb, sr] = forder.astype(np.int32)
        dist_k[sb, sr] = np.take_along_axis(frow, forder, axis=-1)

    valid = dist_k <= CUTOFF

    # edge weight recomputed from positions like the reference
    xc_k = p[bidx, col_k]                                      # [B, N, K, 3]
    diff = p[:, :, None, :] - xc_k
    d0 = diff[..., 0] * diff[..., 0]
    d1 = diff[..., 1] * diff[..., 1]
    d2s = diff[..., 2] * diff[..., 2]
    w = np.sqrt((d0 + d1) + d2s)
    w = np.where(valid, w, np.float32(0.0)).astype(np.float32)

    offs = (np.arange(B, dtype=np.int32) * N)[:, None, None]
    row_g = np.where(valid, rows_l + offs, 0).astype(np.int32)
    col_g = np.where(valid, col_k + offs, 0).astype(np.int32)

    edge_index = np.stack([row_g.reshape(-1), col_g.reshape(-1)], axis=0)
    return edge_index, w.reshape(-1), valid.reshape(-1)


# revision 43
# speedup vs baseline: 1.0076x; 1.0076x over previous
"""DenseRadiusGraph (B=16 graphs x N=2048 nodes, D=3, cutoff=10, K=32)
on 8 Trainium2 NeuronCores.

Strategy
--------
Shard over the graph axis: each of the 8 cores handles 2 graphs; no
cross-core communication.

Per core (device), for each 128-row block of a graph's [N, N] distance
matrix:
  - A K=15 feature GEMM (hi/lo-split bf16: exact bf16 x bf16 products,
    fp32 PSUM accumulate) on the tensor engine computes
        C + z,   z = ZOFF - d2(i, j)/4,   C = 2^23
    directly into PSUM. z lies in [2^14, 2^15), so the fp32 PSUM store
    at 2^23 scale rounds z to the integer grid for free, leaving the
    low 9 mantissa bits clear.
  - One ACT op subtracts C; a gpsimd/DVE column-split add embeds the
    column index j%512 * 2^-9 into the low bits (15+9 = 24 mantissa
    bits: exact).
  - The DVE max8 op extracts, per 512-wide column chunk, the top-8
    values (= 8 smallest d2, index embedded). The radius-graph data
    (cutoff 10 in a 100-box) has ~9 in-cutoff neighbours per row, so a
    chunk's top-8 usually contains every in-cutoff neighbour of that
    chunk; saturated chunks (8th-best still inside the cutoff margin,
    ~830 rows here) are detected on the host and those rows re-solved
    exactly there.

Host post-pass: decode candidate columns, exact re-score with bit-exact
replication of the reference arithmetic (XLA CPU computes the einsum as
an fp32 FMA chain over d=0,1,2, emulated in float64), final top-K +
cutoff + edge emission, plus the exact row-level fallback for saturated
chunks. The O(N^2) distance work and the top-k scan run on device; the
host only re-ranks 64 candidates/row and formats edges.
"""

import numpy as np

import concourse.bass as bass
import concourse.tile as tile
from concourse import bacc, mybir
from concourse.bass_utils import run_bass_kernel_spmd

B = 16          # graphs
N = 2048        # nodes per graph
D = 3
K = 32          # max neighbours emitted
CUTOFF = np.float32(10.0)
NCORES = 8
GPC = B // NCORES            # graphs per core
RB = 128                     # rows per block
NRB = N // RB                # row blocks per graph
CHW = 512                    # selection chunk width
NCH = N // CHW               # chunks per row
CAND = NCH * 8               # candidates per row

GEMM_BF16 = True             # hi/lo-split bf16 GEMM (4x PE throughput)

# The GEMM computes z = ZOFF - d2/4 (features scaled by exact 0.25), so z
# sits in [2^14, 2^15): 15 integer bits + 9 index bits = 24 = fp32 mantissa.
ZOFF = np.float32(24576.0)   # bf16-exact binade offset
D2SCALE = np.float32(4.0)    # d2 = (ZOFF - z) * D2SCALE
CMAGIC = np.float32(8388608.0)    # 2^23: baked into the GEMM so PSUM fp32
                                  # storage rounds z to the integer grid
NFEAT = 15
JEPS = np.float32(1.0 / 512.0)   # index embedding step (2^-9)
SAT_MARGIN = np.float32(16.0)    # d2 slack for saturation/noise margins

DIAG_MASK = False
SPLIT = 1472                 # posc-add column split: [0,SPLIT) gpsimd, rest DVE
NACT = 1                     # ACT -C op split count

_cached = None
last_exec_ns = None


def _build():
    """Build + compile the per-core Bass program (same NEFF for all cores)."""
    nc = bacc.Bacc("TRN2", target_bir_lowering=False, debug=False)
    f32 = mybir.dt.float32
    fdt = mybir.dt.bfloat16

    U = nc.dram_tensor("U", [GPC, NFEAT, N], fdt, kind="ExternalInput")
    V = nc.dram_tensor("V", [GPC, NFEAT, N], fdt, kind="ExternalInput")
    POSC = nc.dram_tensor("POSC", [1, N], f32, kind="ExternalInput")
    vals_out = nc.dram_tensor(
        "vals_out", [GPC, NRB, RB, CAND], f32, kind="ExternalOutput"
    )

    with tile.TileContext(nc) as tc:
        with (
            tc.tile_pool(name="feat", bufs=2) as feat,
            tc.tile_pool(name="posp", bufs=1) as posp,
            tc.tile_pool(name="ps", bufs=2, space="PSUM") as ps,
            tc.tile_pool(name="sb", bufs=2) as sbp,
            tc.tile_pool(name="outp", bufs=3) as outp,
        ):
            posc = posp.tile([RB, N], f32)
            nc.sync.dma_start(posc[:], bass.AP(POSC, 0, [[0, RB], [1, N]]))
            nbias = posp.tile([RB, 1], f32)
            nc.vector.memset(nbias[:], float(-CMAGIC))
            for g in range(GPC):
                u_t = feat.tile([NFEAT, N], fdt, tag="u")
                v_t = feat.tile([NFEAT, N], fdt, tag="v")
                nc.sync.dma_start(u_t[:], U.ap()[g])
                nc.sync.dma_start(v_t[:], V.ap()[g])
                for rb in range(NRB):
                    # PSUM holds C + z already quantized to the integer
                    # grid (fp32 storage at 2^23 scale); ACT subtracts C
                    # back, per half-row so it overlaps the matmuls.
                    zq = sbp.tile([RB, N], f32, tag="zq")
                    for h in range(2):
                        psum = ps.tile([RB, N // 2], f32, tag=f"ps{h}")
                        for cb in range(2):
                            o = h * (N // 2) + cb * 512
                            nc.tensor.matmul(
                                psum[:, cb * 512:(cb + 1) * 512],
                                u_t[:, rb * RB:(rb + 1) * RB],
                                v_t[:, o:o + 512],
                            )
                        nc.scalar.activation(
                            zq[:, h * (N // 2):(h + 1) * (N // 2)], psum[:],
                            mybir.ActivationFunctionType.Identity,
                            bias=nbias[:, 0:1], scale=1.0,
                        )
                    # embed the column index into the low 9 (free) mantissa
                    # bits: v = zq + j*2^-9, split across gpsimd and DVE
                    v_sel = sbp.tile([RB, N], f32, tag="vsel")
                    nc.gpsimd.tensor_tensor(
                        out=v_sel[:, 0:SPLIT], in0=zq[:, 0:SPLIT],
                        in1=posc[:, 0:SPLIT], op=mybir.AluOpType.add,
                    )
                    nc.vector.tensor_tensor(
                        out=v_sel[:, SPLIT:N], in0=zq[:, SPLIT:N],
                        in1=posc[:, SPLIT:N], op=mybir.AluOpType.add,
                    )
                    if DIAG_MASK:
                        nc.gpsimd.affine_select(
                            v_sel[:, rb * RB:(rb + 1) * RB],
                            v_sel[:, rb * RB:(rb + 1) * RB],
                            pattern=[[1, RB]],
                            compare_op=mybir.AluOpType.not_equal,
                            fill=0.0, base=0, channel_multiplier=-1,
                        )
                    vals_t = outp.tile([RB, CAND], f32, tag="vals")
                    for c in range(NCH):
                        nc.vector.max(
                            out=vals_t[:, c * 8:(c + 1) * 8],
                            in_=v_sel[:, c * CHW:(c + 1) * CHW],
                        )
                    nc.sync.dma_start(vals_out.ap()[g, rb], vals_t[:])
    nc.compile()
    return nc


def _get_nc():
    global _cached
    if _cached is None:
        _cached = _build()
    return _cached


def _sq_ref(x):
    """|x|^2 per row, bit-exact to jnp.sum(p*p, -1) on XLA CPU
    (rounded products, sequential adds)."""
    p0 = x[:, 0] * x[:, 0]
    p1 = x[:, 1] * x[:, 1]
    p2 = x[:, 2] * x[:, 2]
    return (p0 + p1) + p2


def _dot_fma(xr, xc):
    """fp32 FMA-chain dot over d=0,1,2, bit-exact to XLA CPU einsum.
    Emulated via float64 (exact product + add, one fp32 rounding)."""
    a = np.float32(xr[..., 0].astype(np.float64) * xc[..., 0].astype(np.float64))
    a = np.float32(
        xr[..., 1].astype(np.float64) * xc[..., 1].astype(np.float64)
        + a.astype(np.float64)
    )
    a = np.float32(
        xr[..., 2].astype(np.float64) * xc[..., 2].astype(np.float64)
        + a.astype(np.float64)
    )
    return a


def _ref_dist(xr, xc, sq_r, sq_c):
    """Reference-bit dist for row points xr [..., 3] vs cols xc [..., 3]."""
    dot = _dot_fma(xr, xc)
    d2 = (sq_r + sq_c) - np.float32(2.0) * dot
    return np.sqrt(np.maximum(d2, np.float32(0.0)))


def _make_in_maps(p, sq):
    import ml_dtypes

    bf16 = ml_dtypes.bfloat16
    posc = ((np.arange(N, dtype=np.float32) % CHW) * JEPS).astype(np.float32)
    in_maps = []
    for core in range(NCORES):
        Uc = np.empty((GPC, NFEAT, N), bf16)
        Vc = np.empty((GPC, NFEAT, N), bf16)
        for g in range(GPC):
            b = core * GPC + g
            x = p[b]                                   # [N, 3] f32
            xh = x.astype(bf16)
            xl = (x - xh.astype(np.float32)).astype(bf16)
            sqh = sq[b].astype(bf16)
            sql = (sq[b] - sqh.astype(np.float32)).astype(bf16)
            # U rows carry the exact pow2 scale 2/D2SCALE = 0.5 (and
            # 1/D2SCALE for the sq rows) so PSUM = C + ZOFF - d2/D2SCALE
            hsc = np.float32(2.0) / D2SCALE
            Uc[g, 0:3] = (hsc * xh.astype(np.float32)).astype(bf16).T
            Uc[g, 3:6] = Uc[g, 0:3]
            Uc[g, 6:9] = (hsc * xl.astype(np.float32)).astype(bf16).T
            Uc[g, 9] = (-sqh.astype(np.float32) / D2SCALE).astype(bf16)
            Uc[g, 10] = (-sql.astype(np.float32) / D2SCALE).astype(bf16)
            Uc[g, 11] = bf16(-1.0 / float(D2SCALE))
            Uc[g, 12] = bf16(-1.0 / float(D2SCALE))
            Uc[g, 13] = bf16(float(ZOFF))
            Uc[g, 14] = bf16(float(CMAGIC))
            Vc[g, 0:3] = xh.T
            Vc[g, 3:6] = xl.T
            Vc[g, 6:9] = xh.T
            Vc[g, 9] = bf16(1.0)
            Vc[g, 10] = bf16(1.0)
            Vc[g, 11] = sqh
            Vc[g, 12] = sql
            Vc[g, 13] = bf16(1.0)
            Vc[g, 14] = bf16(1.0)
        in_maps.append({"U": Uc, "V": Vc, "POSC": posc[None, :].copy()})
    return in_maps


def kernel(pos, batch):
    pos = np.ascontiguousarray(np.asarray(pos), dtype=np.float32)
    nc = _get_nc()

    p = pos.reshape(B, N, D)
    sq = np.stack([_sq_ref(p[b]) for b in range(B)])          # [B, N] f32

    res = run_bass_kernel_spmd(nc, _make_in_maps(p, sq), core_ids=list(range(NCORES)))
    global last_exec_ns
    last_exec_ns = res.exec_time_ns

    vals = np.empty((B, N, CAND), np.float32)
    for core in range(NCORES):
        r = res.results[core]["vals_out"]                     # [GPC, NRB, RB, CAND]
        for g in range(GPC):
            vals[core * GPC + g] = r[g].reshape(N, CAND)

    # decode candidate columns from the index-embedded values
    v64 = vals.astype(np.float64)
    j = np.rint((v64 % 1.0) * CHW).astype(np.int32)
    chunk_base = (np.arange(CAND, dtype=np.int32) // 8) * CHW
    cand = j + chunk_base                                     # [B, N, CAND]

    # saturated chunk detection: 8th-best of a chunk still within the
    # cutoff margin means the chunk may have had >8 in-cutoff columns
    zq = np.floor(v64)                                        # [B, N, CAND]
    d2_approx = (float(ZOFF) - zq) * float(D2SCALE)
    sat = (d2_approx <= float(CUTOFF) ** 2 + float(SAT_MARGIN))
    # slots 7, 15, ... are each chunk's 8th-best
    sat_rows = np.nonzero(sat[:, :, 7::8].any(-1))

    return _host_finish(p, sq, cand, sat_rows)


def _host_finish(p, sq, cand, sat_rows):
    """Exact re-score of candidates with reference-bit arithmetic, final
    top-K + cutoff + edge emission, exact fallback for saturated rows."""
    rows_l = np.arange(N, dtype=np.int32)[None, :, None]       # graph-local row

    bidx = np.arange(B)[:, None, None]
    xr = p[:, :, None, :]                                      # [B, N, 1, 3]
    xc = p[bidx, cand]                                         # [B, N, CAND, 3]
    dist = _ref_dist(xr, xc, sq[:, :, None], sq[bidx, cand])
    dist = np.where(cand == rows_l, np.float32(np.inf), dist)  # drop self-edge

    # sort each row's candidates by (dist, col) ascending — matches
    # jax.lax.top_k's stable lowest-index tie-break
    order = np.lexsort((cand, dist.view(np.uint32)), axis=-1)[:, :, :K]
    col_k = np.take_along_axis(cand, order, axis=-1)           # [B, N, K]
    dist_k = np.take_along_axis(dist, order, axis=-1)

    # exact fallback: saturated rows are re-solved against all N columns
    sb, sr = sat_rows
    if sb.size:
        frow = _ref_dist(p[sb, sr, None, :], p[sb], sq[sb, sr, None], sq[sb])
        frow[np.arange(sb.size), sr] = np.float32(np.inf)      # self
        forder = np.lexsort(
            (np.broadcast_to(np.arange(N, dtype=np.int32), frow.shape),
             frow.view(np.uint32)), axis=-1)[:, :K]
        col_k[sb, sr] = forder.astype(np.int32)
        dist_k[sb, sr] = np.take_along_axis(frow, forder, axis=-1)

    valid = dist_k <= CUTOFF

    # edge weight recomputed from positions like the reference
    xc_k = p[bidx, col_k]                                      # [B, N, K, 3]
    diff = p[:, :, None, :] - xc_k
    d0 = diff[..., 0] * diff[..., 0]
    d1 = diff[..., 1] * diff[..., 1]
    d2s = diff[..., 2] * diff[..., 2]
    w = np.sqrt((d0 + d1) + d2s)
    w = np.where(valid, w, np.float32(0.0)).astype(np.float32)

    offs = (np.arange(B, dtype=np.int32) * N)[:, None, None]
    row_g = np.where(valid, rows_l + offs, 0).astype(np.int32)
    col_g = np.where(valid, col_k + offs, 0).astype(np.int32)

    edge_index = np.stack([row_g.reshape(-1), col_g.reshape(-1)], axis=0)
    return edge_index, w.reshape(-1), valid.reshape(-1)


# revision 44
# speedup vs baseline: 1.0225x; 1.0148x over previous
"""DenseRadiusGraph (B=16 graphs x N=2048 nodes, D=3, cutoff=10, K=32)
on 8 Trainium2 NeuronCores.

Strategy
--------
Shard over the graph axis: each of the 8 cores handles 2 graphs; no
cross-core communication.

Per core (device), for each 128-row block of a graph's [N, N] distance
matrix:
  - A K=15 feature GEMM (hi/lo-split bf16: exact bf16 x bf16 products,
    fp32 PSUM accumulate) on the tensor engine computes
        C + z,   z = ZOFF - d2(i, j)/4,   C = 2^23
    directly into PSUM. z lies in [2^14, 2^15), so the fp32 PSUM store
    at 2^23 scale rounds z to the integer grid for free, leaving the
    low 9 mantissa bits clear.
  - One ACT op subtracts C; a gpsimd/DVE column-split add embeds the
    column index j%512 * 2^-9 into the low bits (15+9 = 24 mantissa
    bits: exact).
  - The DVE max8 op extracts, per 512-wide column chunk, the top-8
    values (= 8 smallest d2, index embedded). The radius-graph data
    (cutoff 10 in a 100-box) has ~9 in-cutoff neighbours per row, so a
    chunk's top-8 usually contains every in-cutoff neighbour of that
    chunk; saturated chunks (8th-best still inside the cutoff margin,
    ~830 rows here) are detected on the host and those rows re-solved
    exactly there.

Host post-pass: decode candidate columns, exact re-score with bit-exact
replication of the reference arithmetic (XLA CPU computes the einsum as
an fp32 FMA chain over d=0,1,2, emulated in float64), final top-K +
cutoff + edge emission, plus the exact row-level fallback for saturated
chunks. The O(N^2) distance work and the top-k scan run on device; the
host only re-ranks 64 candidates/row and formats edges.
"""

import numpy as np

import concourse.bass as bass
import concourse.tile as tile
from concourse import bacc, mybir
from concourse.bass_utils import run_bass_kernel_spmd

B = 16          # graphs
N = 2048        # nodes per graph
D = 3
K = 32          # max neighbours emitted
CUTOFF = np.float32(10.0)
NCORES = 8
GPC = B // NCORES            # graphs per core
RB = 128                     # rows per block
NRB = N // RB                # row blocks per graph
CHW = 512                    # selection chunk width
NCH = N // CHW               # chunks per row
CAND = NCH * 8               # candidates per row

GEMM_BF16 = True             # hi/lo-split bf16 GEMM (4x PE throughput)

# The GEMM computes z = ZOFF - d2/4 (features scaled by exact 0.25), so z
# sits in [2^14, 2^15): 15 integer bits + 9 index bits = 24 = fp32 mantissa.
ZOFF = np.float32(24576.0)   # bf16-exact binade offset
D2SCALE = np.float32(4.0)    # d2 = (ZOFF - z) * D2SCALE
CMAGIC = np.float32(8388608.0)    # 2^23: baked into the GEMM so PSUM fp32
                                  # storage rounds z to the integer grid
NFEAT = 15
JEPS = np.float32(1.0 / 512.0)   # index embedding step (2^-9)
SAT_MARGIN = np.float32(16.0)    # d2 slack for saturation/noise margins

DIAG_MASK = False
SPLIT = 1472                 # posc-add column split: [0,SPLIT) gpsimd, rest DVE
NACT = 1                     # ACT -C op split count

_cached = None
last_exec_ns = None


def _build():
    """Build + compile the per-core Bass program (same NEFF for all cores)."""
    nc = bacc.Bacc("TRN2", target_bir_lowering=False, debug=False)
    f32 = mybir.dt.float32
    fdt = mybir.dt.bfloat16

    U = nc.dram_tensor("U", [GPC, NFEAT, N], fdt, kind="ExternalInput")
    V = nc.dram_tensor("V", [GPC, NFEAT, N], fdt, kind="ExternalInput")
    POSC = nc.dram_tensor("POSC", [1, N], f32, kind="ExternalInput")
    vals_out = nc.dram_tensor(
        "vals_out", [GPC, NRB, RB, CAND], f32, kind="ExternalOutput"
    )

    with tile.TileContext(nc) as tc:
        with (
            tc.tile_pool(name="feat", bufs=2) as feat,
            tc.tile_pool(name="posp", bufs=1) as posp,
            tc.tile_pool(name="ps", bufs=2, space="PSUM") as ps,
            tc.tile_pool(name="sb", bufs=2) as sbp,
            tc.tile_pool(name="outp", bufs=3) as outp,
        ):
            # load posc as one row and broadcast on-device: a [128, N]
            # broadcast DMA (1MB) would serialize ahead of the feature
            # DMAs and stall the pipeline fill by ~3us
            posc = posp.tile([RB, N], f32)
            posc1 = posp.tile([1, N], f32)
            nc.sync.dma_start(posc1[:], POSC.ap())
            nc.gpsimd.partition_broadcast(posc[:], posc1[:])
            nbias = posp.tile([RB, 1], f32)
            nc.vector.memset(nbias[:], float(-CMAGIC))
            for g in range(GPC):
                u_t = feat.tile([NFEAT, N], fdt, tag="u")
                v_t = feat.tile([NFEAT, N], fdt, tag="v")
                nc.sync.dma_start(u_t[:], U.ap()[g])
                nc.sync.dma_start(v_t[:], V.ap()[g])
                for rb in range(NRB):
                    # PSUM holds C + z already quantized to the integer
                    # grid (fp32 storage at 2^23 scale); ACT subtracts C
                    # back, per half-row so it overlaps the matmuls.
                    zq = sbp.tile([RB, N], f32, tag="zq")
                    for h in range(2):
                        psum = ps.tile([RB, N // 2], f32, tag=f"ps{h}")
                        for cb in range(2):
                            o = h * (N // 2) + cb * 512
                            nc.tensor.matmul(
                                psum[:, cb * 512:(cb + 1) * 512],
                                u_t[:, rb * RB:(rb + 1) * RB],
                                v_t[:, o:o + 512],
                            )
                        nc.scalar.activation(
                            zq[:, h * (N // 2):(h + 1) * (N // 2)], psum[:],
                            mybir.ActivationFunctionType.Identity,
                            bias=nbias[:, 0:1], scale=1.0,
                        )
                    # embed the column index into the low 9 (free) mantissa
                    # bits: v = zq + j*2^-9, split across gpsimd and DVE
                    v_sel = sbp.tile([RB, N], f32, tag="vsel")
                    nc.gpsimd.tensor_tensor(
                        out=v_sel[:, 0:SPLIT], in0=zq[:, 0:SPLIT],
                        in1=posc[:, 0:SPLIT], op=mybir.AluOpType.add,
                    )
                    nc.vector.tensor_tensor(
                        out=v_sel[:, SPLIT:N], in0=zq[:, SPLIT:N],
                        in1=posc[:, SPLIT:N], op=mybir.AluOpType.add,
                    )
                    if DIAG_MASK:
                        nc.gpsimd.affine_select(
                            v_sel[:, rb * RB:(rb + 1) * RB],
                            v_sel[:, rb * RB:(rb + 1) * RB],
                            pattern=[[1, RB]],
                            compare_op=mybir.AluOpType.not_equal,
                            fill=0.0, base=0, channel_multiplier=-1,
                        )
                    vals_t = outp.tile([RB, CAND], f32, tag="vals")
                    for c in range(NCH):
                        nc.vector.max(
                            out=vals_t[:, c * 8:(c + 1) * 8],
                            in_=v_sel[:, c * CHW:(c + 1) * CHW],
                        )
                    nc.sync.dma_start(vals_out.ap()[g, rb], vals_t[:])
    nc.compile()
    return nc


def _get_nc():
    global _cached
    if _cached is None:
        _cached = _build()
    return _cached


def _sq_ref(x):
    """|x|^2 per row, bit-exact to jnp.sum(p*p, -1) on XLA CPU
    (rounded products, sequential adds)."""
    p0 = x[:, 0] * x[:, 0]
    p1 = x[:, 1] * x[:, 1]
    p2 = x[:, 2] * x[:, 2]
    return (p0 + p1) + p2


def _dot_fma(xr, xc):
    """fp32 FMA-chain dot over d=0,1,2, bit-exact to XLA CPU einsum.
    Emulated via float64 (exact product + add, one fp32 rounding)."""
    a = np.float32(xr[..., 0].astype(np.float64) * xc[..., 0].astype(np.float64))
    a = np.float32(
        xr[..., 1].astype(np.float64) * xc[..., 1].astype(np.float64)
        + a.astype(np.float64)
    )
    a = np.float32(
        xr[..., 2].astype(np.float64) * xc[..., 2].astype(np.float64)
        + a.astype(np.float64)
    )
    return a


def _ref_dist(xr, xc, sq_r, sq_c):
    """Reference-bit dist for row points xr [..., 3] vs cols xc [..., 3]."""
    dot = _dot_fma(xr, xc)
    d2 = (sq_r + sq_c) - np.float32(2.0) * dot
    return np.sqrt(np.maximum(d2, np.float32(0.0)))


def _make_in_maps(p, sq):
    import ml_dtypes

    bf16 = ml_dtypes.bfloat16
    posc = ((np.arange(N, dtype=np.float32) % CHW) * JEPS).astype(np.float32)
    in_maps = []
    for core in range(NCORES):
        Uc = np.empty((GPC, NFEAT, N), bf16)
        Vc = np.empty((GPC, NFEAT, N), bf16)
        for g in range(GPC):
            b = core * GPC + g
            x = p[b]                                   # [N, 3] f32
            xh = x.astype(bf16)
            xl = (x - xh.astype(np.float32)).astype(bf16)
            sqh = sq[b].astype(bf16)
            sql = (sq[b] - sqh.astype(np.float32)).astype(bf16)
            # U rows carry the exact pow2 scale 2/D2SCALE = 0.5 (and
            # 1/D2SCALE for the sq rows) so PSUM = C + ZOFF - d2/D2SCALE
            hsc = np.float32(2.0) / D2SCALE
            Uc[g, 0:3] = (hsc * xh.astype(np.float32)).astype(bf16).T
            Uc[g, 3:6] = Uc[g, 0:3]
            Uc[g, 6:9] = (hsc * xl.astype(np.float32)).astype(bf16).T
            Uc[g, 9] = (-sqh.astype(np.float32) / D2SCALE).astype(bf16)
            Uc[g, 10] = (-sql.astype(np.float32) / D2SCALE).astype(bf16)
            Uc[g, 11] = bf16(-1.0 / float(D2SCALE))
            Uc[g, 12] = bf16(-1.0 / float(D2SCALE))
            Uc[g, 13] = bf16(float(ZOFF))
            Uc[g, 14] = bf16(float(CMAGIC))
            Vc[g, 0:3] = xh.T
            Vc[g, 3:6] = xl.T
            Vc[g, 6:9] = xh.T
            Vc[g, 9] = bf16(1.0)
            Vc[g, 10] = bf16(1.0)
            Vc[g, 11] = sqh
            Vc[g, 12] = sql
            Vc[g, 13] = bf16(1.0)
            Vc[g, 14] = bf16(1.0)
        in_maps.append({"U": Uc, "V": Vc, "POSC": posc[None, :].copy()})
    return in_maps


def kernel(pos, batch):
    pos = np.ascontiguousarray(np.asarray(pos), dtype=np.float32)
    nc = _get_nc()

    p = pos.reshape(B, N, D)
    sq = np.stack([_sq_ref(p[b]) for b in range(B)])          # [B, N] f32

    res = run_bass_kernel_spmd(nc, _make_in_maps(p, sq), core_ids=list(range(NCORES)))
    global last_exec_ns
    last_exec_ns = res.exec_time_ns

    vals = np.empty((B, N, CAND), np.float32)
    for core in range(NCORES):
        r = res.results[core]["vals_out"]                     # [GPC, NRB, RB, CAND]
        for g in range(GPC):
            vals[core * GPC + g] = r[g].reshape(N, CAND)

    # decode candidate columns from the index-embedded values
    v64 = vals.astype(np.float64)
    j = np.rint((v64 % 1.0) * CHW).astype(np.int32)
    chunk_base = (np.arange(CAND, dtype=np.int32) // 8) * CHW
    cand = j + chunk_base                                     # [B, N, CAND]

    # saturated chunk detection: 8th-best of a chunk still within the
    # cutoff margin means the chunk may have had >8 in-cutoff columns
    zq = np.floor(v64)                                        # [B, N, CAND]
    d2_approx = (float(ZOFF) - zq) * float(D2SCALE)
    sat = (d2_approx <= float(CUTOFF) ** 2 + float(SAT_MARGIN))
    # slots 7, 15, ... are each chunk's 8th-best
    sat_rows = np.nonzero(sat[:, :, 7::8].any(-1))

    return _host_finish(p, sq, cand, sat_rows)


def _host_finish(p, sq, cand, sat_rows):
    """Exact re-score of candidates with reference-bit arithmetic, final
    top-K + cutoff + edge emission, exact fallback for saturated rows."""
    rows_l = np.arange(N, dtype=np.int32)[None, :, None]       # graph-local row

    bidx = np.arange(B)[:, None, None]
    xr = p[:, :, None, :]                                      # [B, N, 1, 3]
    xc = p[bidx, cand]                                         # [B, N, CAND, 3]
    dist = _ref_dist(xr, xc, sq[:, :, None], sq[bidx, cand])
    dist = np.where(cand == rows_l, np.float32(np.inf), dist)  # drop self-edge

    # sort each row's candidates by (dist, col) ascending — matches
    # jax.lax.top_k's stable lowest-index tie-break
    order = np.lexsort((cand, dist.view(np.uint32)), axis=-1)[:, :, :K]
    col_k = np.take_along_axis(cand, order, axis=-1)           # [B, N, K]
    dist_k = np.take_along_axis(dist, order, axis=-1)

    # exact fallback: saturated rows are re-solved against all N columns
    sb, sr = sat_rows
    if sb.size:
        frow = _ref_dist(p[sb, sr, None, :], p[sb], sq[sb, sr, None], sq[sb])
        frow[np.arange(sb.size), sr] = np.float32(np.inf)      # self
        forder = np.lexsort(
            (np.broadcast_to(np.arange(N, dtype=np.int32), frow.shape),
             frow.view(np.uint32)), axis=-1)[:, :K]
        col_k[sb, sr] = forder.astype(np.int32)
        dist_k[sb, sr] = np.take_along_axis(frow, forder, axis=-1)

    valid = dist_k <= CUTOFF

    # edge weight recomputed from positions like the reference
    xc_k = p[bidx, col_k]                                      # [B, N, K, 3]
    diff = p[:, :, None, :] - xc_k
    d0 = diff[..., 0] * diff[..., 0]
    d1 = diff[..., 1] * diff[..., 1]
    d2s = diff[..., 2] * diff[..., 2]
    w = np.sqrt((d0 + d1) + d2s)
    w = np.where(valid, w, np.float32(0.0)).astype(np.float32)

    offs = (np.arange(B, dtype=np.int32) * N)[:, None, None]
    row_g = np.where(valid, rows_l + offs, 0).astype(np.int32)
    col_g = np.where(valid, col_k + offs, 0).astype(np.int32)

    edge_index = np.stack([row_g.reshape(-1), col_g.reshape(-1)], axis=0)
    return edge_index, w.reshape(-1), valid.reshape(-1)


# revision 45
# speedup vs baseline: 1.0327x; 1.0100x over previous
"""DenseRadiusGraph (B=16 graphs x N=2048 nodes, D=3, cutoff=10, K=32)
on 8 Trainium2 NeuronCores.

Strategy
--------
Shard over the graph axis: each of the 8 cores handles 2 graphs; no
cross-core communication.

Per core (device), for each 128-row block of a graph's [N, N] distance
matrix:
  - A K=15 feature GEMM (hi/lo-split bf16: exact bf16 x bf16 products,
    fp32 PSUM accumulate) on the tensor engine computes
        C + z,   z = ZOFF - d2(i, j)/4,   C = 2^23
    directly into PSUM. z lies in [2^14, 2^15), so the fp32 PSUM store
    at 2^23 scale rounds z to the integer grid for free, leaving the
    low 9 mantissa bits clear.
  - One ACT op subtracts C; a gpsimd/DVE column-split add embeds the
    column index j%512 * 2^-9 into the low bits (15+9 = 24 mantissa
    bits: exact).
  - The DVE max8 op extracts, per 512-wide column chunk, the top-8
    values (= 8 smallest d2, index embedded). The radius-graph data
    (cutoff 10 in a 100-box) has ~9 in-cutoff neighbours per row, so a
    chunk's top-8 usually contains every in-cutoff neighbour of that
    chunk; saturated chunks (8th-best still inside the cutoff margin,
    ~830 rows here) are detected on the host and those rows re-solved
    exactly there.

Host post-pass: decode candidate columns, exact re-score with bit-exact
replication of the reference arithmetic (XLA CPU computes the einsum as
an fp32 FMA chain over d=0,1,2, emulated in float64), final top-K +
cutoff + edge emission, plus the exact row-level fallback for saturated
chunks. The O(N^2) distance work and the top-k scan run on device; the
host only re-ranks 64 candidates/row and formats edges.
"""

import numpy as np

import concourse.bass as bass
import concourse.tile as tile
from concourse import bacc, mybir
from concourse.bass_utils import run_bass_kernel_spmd

B = 16          # graphs
N = 2048        # nodes per graph
D = 3
K = 32          # max neighbours emitted
CUTOFF = np.float32(10.0)
NCORES = 8
GPC = B // NCORES            # graphs per core
RB = 128                     # rows per block
NRB = N // RB                # row blocks per graph
CHW = 512                    # selection chunk width
NCH = N // CHW               # chunks per row
CAND = NCH * 8               # candidates per row

GEMM_BF16 = True             # hi/lo-split bf16 GEMM (4x PE throughput)

# The GEMM computes z = ZOFF - d2/4 (features scaled by exact 0.25), so z
# sits in [2^14, 2^15): 15 integer bits + 9 index bits = 24 = fp32 mantissa.
ZOFF = np.float32(24576.0)   # bf16-exact binade offset
D2SCALE = np.float32(4.0)    # d2 = (ZOFF - z) * D2SCALE
CMAGIC = np.float32(8388608.0)    # 2^23: baked into the GEMM so PSUM fp32
                                  # storage rounds z to the integer grid
NFEAT = 15
JEPS = np.float32(1.0 / 512.0)   # index embedding step (2^-9)
SAT_MARGIN = np.float32(16.0)    # d2 slack for saturation/noise margins

DIAG_MASK = False
SPLIT = 1472                 # posc-add column split: [0,SPLIT) gpsimd, rest DVE
NACT = 1                     # ACT -C op split count

_cached = None
last_exec_ns = None


def _build():
    """Build + compile the per-core Bass program (same NEFF for all cores)."""
    nc = bacc.Bacc("TRN2", target_bir_lowering=False, debug=False)
    f32 = mybir.dt.float32
    fdt = mybir.dt.bfloat16

    U = nc.dram_tensor("U", [GPC, NFEAT, N], fdt, kind="ExternalInput")
    V = nc.dram_tensor("V", [GPC, NFEAT, N], fdt, kind="ExternalInput")
    POSC = nc.dram_tensor("POSC", [1, N], f32, kind="ExternalInput")
    vals_out = nc.dram_tensor(
        "vals_out", [GPC, NRB, RB, CAND], f32, kind="ExternalOutput"
    )

    with tile.TileContext(nc) as tc:
        with (
            tc.tile_pool(name="feat", bufs=2) as feat,
            tc.tile_pool(name="posp", bufs=1) as posp,
            tc.tile_pool(name="ps", bufs=2, space="PSUM") as ps,
            tc.tile_pool(name="sb", bufs=2) as sbp,
            tc.tile_pool(name="outp", bufs=3) as outp,
        ):
            # load posc as one row and broadcast on-device: a [128, N]
            # broadcast DMA (1MB) would serialize ahead of the feature
            # DMAs and stall the pipeline fill by ~3us
            posc = posp.tile([RB, N], f32)
            posc1 = posp.tile([1, N], f32)
            nc.sync.dma_start(posc1[:], POSC.ap())
            nc.gpsimd.partition_broadcast(posc[:], posc1[:])
            nbias = posp.tile([RB, 1], f32)
            nc.vector.memset(nbias[:], float(-CMAGIC))
            for g in range(GPC):
                u_t = feat.tile([NFEAT, N], fdt, tag="u")
                v_t = feat.tile([NFEAT, N], fdt, tag="v")
                nc.sync.dma_start(u_t[:], U.ap()[g])
                nc.sync.dma_start(v_t[:], V.ap()[g])
                for rb in range(NRB):
                    # PSUM holds C + z already quantized to the integer
                    # grid (fp32 storage at 2^23 scale); ACT subtracts C
                    # back, per half-row so it overlaps the matmuls.
                    zq = sbp.tile([RB, N], f32, tag="zq")
                    for h in range(2):
                        psum = ps.tile([RB, N // 2], f32, tag=f"ps{h}")
                        for cb in range(2):
                            o = h * (N // 2) + cb * 512
                            nc.tensor.matmul(
                                psum[:, cb * 512:(cb + 1) * 512],
                                u_t[:, rb * RB:(rb + 1) * RB],
                                v_t[:, o:o + 512],
                            )
                        nc.scalar.activation(
                            zq[:, h * (N // 2):(h + 1) * (N // 2)], psum[:],
                            mybir.ActivationFunctionType.Identity,
                            bias=nbias[:, 0:1], scale=1.0,
                        )
                    # embed the column index into the low 9 (free) mantissa
                    # bits: v = zq + j*2^-9, split across gpsimd and DVE.
                    # The very first tile gives most columns to DVE: at
                    # pipeline fill the gpsimd add would otherwise sit on
                    # the critical path before DVE has any work.
                    sp = 512 if (g == 0 and rb == 0) else SPLIT
                    v_sel = sbp.tile([RB, N], f32, tag="vsel")
                    nc.gpsimd.tensor_tensor(
                        out=v_sel[:, 0:sp], in0=zq[:, 0:sp],
                        in1=posc[:, 0:sp], op=mybir.AluOpType.add,
                    )
                    nc.vector.tensor_tensor(
                        out=v_sel[:, sp:N], in0=zq[:, sp:N],
                        in1=posc[:, sp:N], op=mybir.AluOpType.add,
                    )
                    if DIAG_MASK:
                        nc.gpsimd.affine_select(
                            v_sel[:, rb * RB:(rb + 1) * RB],
                            v_sel[:, rb * RB:(rb + 1) * RB],
                            pattern=[[1, RB]],
                            compare_op=mybir.AluOpType.not_equal,
                            fill=0.0, base=0, channel_multiplier=-1,
                        )
                    vals_t = outp.tile([RB, CAND], f32, tag="vals")
                    for c in range(NCH):
                        nc.vector.max(
                            out=vals_t[:, c * 8:(c + 1) * 8],
                            in_=v_sel[:, c * CHW:(c + 1) * CHW],
                        )
                    nc.sync.dma_start(vals_out.ap()[g, rb], vals_t[:])
    nc.compile()
    return nc


def _get_nc():
    global _cached
    if _cached is None:
        _cached = _build()
    return _cached


def _sq_ref(x):
    """|x|^2 per row, bit-exact to jnp.sum(p*p, -1) on XLA CPU
    (rounded products, sequential adds)."""
    p0 = x[:, 0] * x[:, 0]
    p1 = x[:, 1] * x[:, 1]
    p2 = x[:, 2] * x[:, 2]
    return (p0 + p1) + p2


def _dot_fma(xr, xc):
    """fp32 FMA-chain dot over d=0,1,2, bit-exact to XLA CPU einsum.
    Emulated via float64 (exact product + add, one fp32 rounding)."""
    a = np.float32(xr[..., 0].astype(np.float64) * xc[..., 0].astype(np.float64))
    a = np.float32(
        xr[..., 1].astype(np.float64) * xc[..., 1].astype(np.float64)
        + a.astype(np.float64)
    )
    a = np.float32(
        xr[..., 2].astype(np.float64) * xc[..., 2].astype(np.float64)
        + a.astype(np.float64)
    )
    return a


def _ref_dist(xr, xc, sq_r, sq_c):
    """Reference-bit dist for row points xr [..., 3] vs cols xc [..., 3]."""
    dot = _dot_fma(xr, xc)
    d2 = (sq_r + sq_c) - np.float32(2.0) * dot
    return np.sqrt(np.maximum(d2, np.float32(0.0)))


def _make_in_maps(p, sq):
    import ml_dtypes

    bf16 = ml_dtypes.bfloat16
    posc = ((np.arange(N, dtype=np.float32) % CHW) * JEPS).astype(np.float32)
    in_maps = []
    for core in range(NCORES):
        Uc = np.empty((GPC, NFEAT, N), bf16)
        Vc = np.empty((GPC, NFEAT, N), bf16)
        for g in range(GPC):
            b = core * GPC + g
            x = p[b]                                   # [N, 3] f32
            xh = x.astype(bf16)
            xl = (x - xh.astype(np.float32)).astype(bf16)
            sqh = sq[b].astype(bf16)
            sql = (sq[b] - sqh.astype(np.float32)).astype(bf16)
            # U rows carry the exact pow2 scale 2/D2SCALE = 0.5 (and
            # 1/D2SCALE for the sq rows) so PSUM = C + ZOFF - d2/D2SCALE
            hsc = np.float32(2.0) / D2SCALE
            Uc[g, 0:3] = (hsc * xh.astype(np.float32)).astype(bf16).T
            Uc[g, 3:6] = Uc[g, 0:3]
            Uc[g, 6:9] = (hsc * xl.astype(np.float32)).astype(bf16).T
            Uc[g, 9] = (-sqh.astype(np.float32) / D2SCALE).astype(bf16)
            Uc[g, 10] = (-sql.astype(np.float32) / D2SCALE).astype(bf16)
            Uc[g, 11] = bf16(-1.0 / float(D2SCALE))
            Uc[g, 12] = bf16(-1.0 / float(D2SCALE))
            Uc[g, 13] = bf16(float(ZOFF))
            Uc[g, 14] = bf16(float(CMAGIC))
            Vc[g, 0:3] = xh.T
            Vc[g, 3:6] = xl.T
            Vc[g, 6:9] = xh.T
            Vc[g, 9] = bf16(1.0)
            Vc[g, 10] = bf16(1.0)
            Vc[g, 11] = sqh
            Vc[g, 12] = sql
            Vc[g, 13] = bf16(1.0)
            Vc[g, 14] = bf16(1.0)
        in_maps.append({"U": Uc, "V": Vc, "POSC": posc[None, :].copy()})
    return in_maps


def kernel(pos, batch):
    pos = np.ascontiguousarray(np.asarray(pos), dtype=np.float32)
    nc = _get_nc()

    p = pos.reshape(B, N, D)
    sq = np.stack([_sq_ref(p[b]) for b in range(B)])          # [B, N] f32

    res = run_bass_kernel_spmd(nc, _make_in_maps(p, sq), core_ids=list(range(NCORES)))
    global last_exec_ns
    last_exec_ns = res.exec_time_ns

    vals = np.empty((B, N, CAND), np.float32)
    for core in range(NCORES):
        r = res.results[core]["vals_out"]                     # [GPC, NRB, RB, CAND]
        for g in range(GPC):
            vals[core * GPC + g] = r[g].reshape(N, CAND)

    # decode candidate columns from the index-embedded values
    v64 = vals.astype(np.float64)
    j = np.rint((v64 % 1.0) * CHW).astype(np.int32)
    chunk_base = (np.arange(CAND, dtype=np.int32) // 8) * CHW
    cand = j + chunk_base                                     # [B, N, CAND]

    # saturated chunk detection: 8th-best of a chunk still within the
    # cutoff margin means the chunk may have had >8 in-cutoff columns
    zq = np.floor(v64)                                        # [B, N, CAND]
    d2_approx = (float(ZOFF) - zq) * float(D2SCALE)
    sat = (d2_approx <= float(CUTOFF) ** 2 + float(SAT_MARGIN))
    # slots 7, 15, ... are each chunk's 8th-best
    sat_rows = np.nonzero(sat[:, :, 7::8].any(-1))

    return _host_finish(p, sq, cand, sat_rows)


def _host_finish(p, sq, cand, sat_rows):
    """Exact re-score of candidates with reference-bit arithmetic, final
    top-K + cutoff + edge emission, exact fallback for saturated rows."""
    rows_l = np.arange(N, dtype=np.int32)[None, :, None]       # graph-local row

    bidx = np.arange(B)[:, None, None]
    xr = p[:, :, None, :]                                      # [B, N, 1, 3]
    xc = p[bidx, cand]                                         # [B, N, CAND, 3]
    dist = _ref_dist(xr, xc, sq[:, :, None], sq[bidx, cand])
    dist = np.where(cand == rows_l, np.float32(np.inf), dist)  # drop self-edge

    # sort each row's candidates by (dist, col) ascending — matches
    # jax.lax.top_k's stable lowest-index tie-break
    order = np.lexsort((cand, dist.view(np.uint32)), axis=-1)[:, :, :K]
    col_k = np.take_along_axis(cand, order, axis=-1)           # [B, N, K]
    dist_k = np.take_along_axis(dist, order, axis=-1)

    # exact fallback: saturated rows are re-solved against all N columns
    sb, sr = sat_rows
    if sb.size:
        frow = _ref_dist(p[sb, sr, None, :], p[sb], sq[sb, sr, None], sq[sb])
        frow[np.arange(sb.size), sr] = np.float32(np.inf)      # self
        forder = np.lexsort(
            (np.broadcast_to(np.arange(N, dtype=np.int32), frow.shape),
             frow.view(np.uint32)), axis=-1)[:, :K]
        col_k[sb, sr] = forder.astype(np.int32)
        dist_k[sb, sr] = np.take_along_axis(frow, forder, axis=-1)

    valid = dist_k <= CUTOFF

    # edge weight recomputed from positions like the reference
    xc_k = p[bidx, col_k]                                      # [B, N, K, 3]
    diff = p[:, :, None, :] - xc_k
    d0 = diff[..., 0] * diff[..., 0]
    d1 = diff[..., 1] * diff[..., 1]
    d2s = diff[..., 2] * diff[..., 2]
    w = np.sqrt((d0 + d1) + d2s)
    w = np.where(valid, w, np.float32(0.0)).astype(np.float32)

    offs = (np.arange(B, dtype=np.int32) * N)[:, None, None]
    row_g = np.where(valid, rows_l + offs, 0).astype(np.int32)
    col_g = np.where(valid, col_k + offs, 0).astype(np.int32)

    edge_index = np.stack([row_g.reshape(-1), col_g.reshape(-1)], axis=0)
    return edge_index, w.reshape(-1), valid.reshape(-1)
